# revision 1
# baseline (speedup 1.0000x reference)
"""Trainium2 Bass kernel: separable 25-tap Gaussian blur (sigma=4) on
[1, 3, 4096, 4096] f32 with edge-replicate padding.

reference computes  blur(img/img.max()) * img.max();  conv is linear, so this
equals blur(img) up to f32 rounding (~1e-7) -- the global max is skipped.

Scheme (per core, H sharded 8 ways into 512-row slabs + 12-row halos):
  * host: edge-pad to [3, 4120, 4120] fp16, slice 536-row slabs per core
  * vertical pass:  fused conv+transpose matmuls. For each 128-wide w-slice j,
    out_V[w, h_out 0..511] = sum_t  X_t[:, wsl].T @ M_t   (PSUM accumulate
    over 5 input row-tiles t with banded constant matrices M_t).
    Result Ys_j = [w=128 partitions, h=512] -- transposed layout.
  * horizontal pass: identical structure on Ys (contraction now over w),
    which transposes back: out2 = [h=128, w_out] natural layout.
  * PSUM evacuated by DVE (vertical) and ACT (horizontal), DMA out.

Compute dtype fp16 (PE 1 cy/row; fp32 would be 4), accumulation fp32 in PSUM.
Measured (numpy sim) absmax error vs f32 reference ~6e-4 of scale.
"""

import json
import sys

import numpy as np

SIGMA = 4.0
HALF = 12
KSZ = 25
H, W, C = 4096, 4096, 3
N_CORES = 8
SLAB = H // N_CORES          # 512 output rows per core
PAD_W = W + 2 * HALF         # 4120
N_WTILES = 33                # ceil(4120 / 128); last tile 24 wide
WINDOWS = [(0, 128), (104, 256), (232, 384), (360, 512), (488, 512)]
OUT_DT_NP = np.float16       # output staged in fp16, upcast on host

_PATCHED = False
_NC_CACHE = {}


def _patch_bass_for_this_walrus():
    """This container's walrus encodes at most ONE inline sem wait per
    instruction ("Too many sync wait commands" otherwise).  Tile freely puts
    several waits on one instruction, so rewrite the BIR JSON at serialization
    time: hoist every multi-wait into standalone EventSemaphore instructions
    (the encoding `wait_ge` uses, which this walrus accepts) placed just
    before the instruction on the same engine queue."""
    global _PATCHED
    if _PATCHED:
        return
    import concourse.bass as bass

    orig = bass.Bass.to_json_bytes

    def _split_multi_waits(self):
        raw = orig(self)
        bir = json.loads(raw)
        ctr = 0
        changed = False
        for fn in bir.get("functions", []):
            for blk in fn.get("blocks", []):
                insts = blk.get("instructions")
                if not insts:
                    continue
                new = []
                for ins in insts:
                    si = ins.get("sync_info")
                    waits = (si or {}).get("on_wait") or []
                    if len(waits) > 1:
                        changed = True
                        for w in waits:
                            ctr += 1
                            ev = {
                                "engine": ins["engine"],
                                "ins": [],
                                "outs": [],
                                "name": f"mwsplit_{ctr}_{ins.get('name', '')}",
                                "opcode": "EventSemaphore",
                                "sync_info": {"on_update": [], "on_wait": [w]},
                            }
                            if "debug" in ins:
                                ev["debug"] = ins["debug"]
                            new.append(ev)
                        si["on_wait"] = []
                    new.append(ins)
                blk["instructions"] = new
        if not changed:
            return raw
        return json.dumps(bir).encode()

    bass.Bass.to_json_bytes = _split_multi_waits
    _PATCHED = True


def _gauss_1d():
    x = np.arange(-HALF, HALF + 1, dtype=np.float64)
    k = np.exp(-0.5 * (x / SIGMA) ** 2)
    return k / k.sum()


def _band_matrices(dtype=np.float16):
    k = _gauss_1d()
    mf = np.zeros((128, 128), np.float64)
    for p in range(128):
        for n in range(max(0, p - 24), p + 1):
            mf[p, n] = k[p - n]
    mm = np.zeros((128, 152), np.float64)
    for p in range(128):
        for n in range(p, min(152, p + 25)):
            mm[p, n] = k[p - n + 24]
    ml = np.zeros((24, 24), np.float64)
    for p in range(24):
        for n in range(p, 24):
            ml[p, n] = k[p - n + 24]
    return mf.astype(dtype), mm.astype(dtype), ml.astype(dtype)


def _build_nc():
    """Build the per-core SPMD Bass program (all 8 cores run the same code on
    different slabs)."""
    _patch_bass_for_this_walrus()
    import concourse.bass as bass
    import concourse.tile as tile
    from concourse import mybir
    from contextlib import ExitStack

    f16 = mybir.dt.float16
    f32 = mybir.dt.float32
    out_dt = f16 if OUT_DT_NP == np.float16 else f32

    mf_np, mm_np, ml_np = _band_matrices(np.float16)

    nc = bass.Bass()
    x = nc.declare_dram_parameter("x", [C, 640, PAD_W], f16, isOutput=False)
    y = nc.declare_dram_parameter("y", [C, SLAB, W], out_dt, isOutput=True)
    mf_d = nc.inline_tensor(mf_np, name="mf")
    mm_d = nc.inline_tensor(mm_np, name="mm")
    ml_d = nc.inline_tensor(ml_np, name="ml")

    with tile.TileContext(nc) as tc, ExitStack() as ctx:
        consts = ctx.enter_context(tc.tile_pool(name="consts", bufs=1))
        xpool = ctx.enter_context(tc.tile_pool(name="xp", bufs=2))
        yspool = ctx.enter_context(tc.tile_pool(name="ys", bufs=2))
        opool = ctx.enter_context(tc.tile_pool(name="ostage", bufs=2))
        psv = ctx.enter_context(tc.tile_pool(name="psv", bufs=3, space="PSUM"))
        psh = ctx.enter_context(tc.tile_pool(name="psh", bufs=3, space="PSUM"))

        mf = consts.tile([128, 128], f16)
        nc.sync.dma_start(mf[:], mf_d[:])
        mm = consts.tile([128, 152], f16)
        nc.sync.dma_start(mm[:], mm_d[:])
        ml = consts.tile([24, 24], f16)
        nc.sync.dma_start(ml[:], ml_d[:])
        mats = [mf, mm, mm, mm, ml]

        for c in range(C):
            xt = xpool.tile([128, 5, PAD_W], f16)
            nc.sync.dma_start(xt[:], x[c].rearrange("(t p) w -> p t w", p=128))

            ys = yspool.tile([128, N_WTILES, 512], f16)

            # vertical pass (conv over h, output transposed to [w, h])
            for j in range(N_WTILES):
                m = 128 if j < N_WTILES - 1 else PAD_W - 128 * (N_WTILES - 1)
                pv = psv.tile([128, 512], f32)
                for t in range(5):
                    n0, n1 = WINDOWS[t]
                    kp = 128 if t < 4 else 24
                    nc.tensor.matmul(
                        out=pv[0:m, n0:n1],
                        lhsT=xt[0:kp, t, 128 * j : 128 * j + m],
                        rhs=mats[t][0:kp, 0 : n1 - n0],
                        start=(t == 0),
                        stop=(t == 4),
                    )
                nc.vector.tensor_copy(ys[0:m, j, :], pv[0:m, :])

            # horizontal pass (conv over w, transposes back to [h, w])
            for b2 in range(2):
                ot = opool.tile([128, 2, W], out_dt)
                for bi in range(2):
                    b = 2 * b2 + bi
                    for q in range(W // 512):
                        ph = psh.tile([128, 512], f32)
                        for t in range(5):
                            j = 4 * q + t
                            n0, n1 = WINDOWS[t]
                            kp = 128 if (t < 4 and j < N_WTILES - 1) else 24
                            nc.tensor.matmul(
                                out=ph[:, n0:n1],
                                lhsT=ys[0:kp, j, 128 * b : 128 * b + 128],
                                rhs=mats[t][0:kp, 0 : n1 - n0],
                                start=(t == 0),
                                stop=(t == 4),
                            )
                        nc.scalar.copy(ot[:, bi, 512 * q : 512 * q + 512], ph[:, :])
                nc.sync.dma_start(
                    y[c, 256 * b2 : 256 * b2 + 256, :].rearrange(
                        "(b p) w -> p b w", p=128
                    ),
                    ot[:],
                )
    return nc


def _get_nc():
    if "nc" not in _NC_CACHE:
        _NC_CACHE["nc"] = _build_nc()
    return _NC_CACHE["nc"]


def _shard_inputs(img):
    """img [1,3,4096,4096] f32 -> per-core padded fp16 slabs [3,640,4120]."""
    x = np.asarray(img)[0]
    xh = x.astype(np.float16)
    xp = np.pad(xh, ((0, 0), (HALF, HALF), (HALF, HALF)), mode="edge")
    in_maps = []
    for core in range(N_CORES):
        buf = np.zeros((C, 640, PAD_W), np.float16)
        buf[:, : SLAB + 2 * HALF] = xp[:, SLAB * core : SLAB * core + SLAB + 2 * HALF]
        in_maps.append({"x": buf})
    return in_maps


def kernel(img):
    from concourse.bass_utils import run_bass_kernel_spmd

    nc = _get_nc()
    in_maps = _shard_inputs(img)
    core_ids = list(range(N_CORES))

    import os

    trace = bool(os.environ.get("KNN_TRACE"))
    res = run_bass_kernel_spmd(nc, in_maps, core_ids, trace=trace)
    _NC_CACHE["last_exec_time_ns"] = res.exec_time_ns
    _NC_CACHE["last_results"] = res

    out = np.empty((C, H, W), np.float32)
    for core in core_ids:
        out[:, SLAB * core : SLAB * (core + 1), :] = res.results[core]["y"].astype(
            np.float32
        )
    return out


if __name__ == "__main__":
    # native compile smoke (no hardware)
    import tempfile
    from concourse.bass_utils import compile_bass_kernel

    nc = _build_nc()
    with tempfile.TemporaryDirectory() as td:
        neff = compile_bass_kernel(nc, td)
        print("COMPILED OK:", neff)

